# revision 5
# baseline (speedup 1.0000x reference)
"""Trainium2 Bass kernel for nn_Model_17789754540645 (dense transformer
attention block: qkv -> per-head softmax(q k^T * sqrt(hd)) v -> proj).

Sharding (8 cores): data-parallel over batch (2) x tensor-parallel over heads
(16 heads -> 4 per core). Each core computes qkv for its 4 heads, full
attention for those heads, and a partial proj output (row-sharded Wproj);
the host sums the 4 partials per batch (all-reduce done host-side) and adds
bproj.

Self-contained: hardcodes shapes; only needs the container's concourse stack.
"""

import sys
import numpy as np

for _p in ("/opt/trn_rl_repo", "/opt/pypackages"):
    if _p not in sys.path:
        sys.path.append(_p)

import concourse.bass as bass
import concourse.tile as tile
from concourse import mybir
from concourse.masks import make_identity
from concourse.vector_clock import ScopedClock, VectorClock

F32 = mybir.dt.float32
F32R = mybir.dt.float32r
BF16 = mybir.dt.bfloat16
AX = mybir.AxisListType
OP = mybir.AluOpType
ACTF = mybir.ActivationFunctionType

B, S_FULL, E, H, HD = 2, 2048, 2048, 16, 128
N_CORES = 8
HLOC_FULL = H // (N_CORES // B)  # 4 heads per core


# ---------------------------------------------------------------------------
# Walrus workaround: this container's walrus rejects >1 semaphore wait on
# several instruction encodings. Split extra waits onto single-wait NoOps.
# ---------------------------------------------------------------------------
_split_counter = [0]


def _split_multi_waits(nc, max_waits=1):
    n = 0
    for fn in nc.m.functions:
        for bb in fn.blocks:
            out, changed = [], False
            for inst in bb.instructions:
                si = inst.sync_info
                waits = list(si.on_wait) if (si and si.on_wait) else []
                if len(waits) > max_waits:
                    changed = True
                    extra, keep = waits[:-max_waits], waits[-max_waits:]
                    for w in extra:
                        _split_counter[0] += 1
                        nop = mybir.InstNoOp(
                            name=f"I-wsplit-{_split_counter[0]}", ins=[], outs=[]
                        )
                        nop.engine = inst.engine
                        nop.sync_info = mybir.SyncInfo(on_wait=[w], on_update=[])
                        out.append(nop)
                        n += 1
                    inst.sync_info = mybir.SyncInfo(
                        on_wait=keep,
                        on_update=list(si.on_update) if si.on_update else [],
                    )
                out.append(inst)
            if changed:
                bb.instructions = out
    return n


def _drain_and_barrier_split(self, tick_clock, wait_clock):
    """Replacement for TileContext._drain_and_barrier emitting <=1 wait per
    instruction (stock version puts every outstanding sem wait on one Drain,
    which this walrus rejects)."""
    gc = tick_clock.global_clock
    n = len(gc)
    active = [i for i in range(n) if gc[i] > 0]
    observed = ScopedClock({None: VectorClock([0] * n)})
    for i in active:
        vals = [gc[j] if j == i else 0 for j in range(n)]
        partial = ScopedClock({None: VectorClock(vals)})
        nop_inst = self.nc.sync.nop(nofuse=True)
        wait_clock.add_sem_waits(nop_inst.ins, partial, observed)
        observed.update_past(partial)
    drain_inst = self.nc.sync.drain()
    wait_clock.add_sem_waits(drain_inst.ins, ScopedClock({None: gc}), observed)

    self.nc.all_engine_barrier()
    assert self.sems is not None
    popped = self.nc._tile_sem_poison_stack.pop()
    assert popped is self._sem_poison
    self.nc.clear_and_free_semaphores(list(self.sems.allocated().values()))
    self.nc.all_engine_barrier()


tile.TileContext._drain_and_barrier = _drain_and_barrier_split


# ---------------------------------------------------------------------------
# Device program (SPMD - same program on all 8 cores, per-core inputs differ)
# ---------------------------------------------------------------------------

def build_program(S=S_FULL, HLOC=HLOC_FULL):
    NEC = E // 128          # 16 e-chunks (contraction for qkv)
    NSB = S // 512          # s blocks of 512
    NST = S // 128          # s tiles of 128
    NFT = 2 * HLOC          # qk feature tiles of 128 (q0 k0 q1 k1 ...)
    NQT = S // 128          # q tiles
    NKB = S // 512          # k blocks of 512
    NKT = S // 128          # k tiles of 128
    NQB = S // 512          # q blocks of 512
    NEB = E // 512          # output eo blocks
    VW = HLOC * 128         # v width (512 at full size)

    nc = bass.Bass()
    xt_p = nc.declare_dram_parameter("xt", [E, S], F32R, isOutput=False)
    wqk_p = nc.declare_dram_parameter("wqk", [NFT, 128, E], F32R, isOutput=False)
    wv_p = nc.declare_dram_parameter("wv", [E, VW], F32R, isOutput=False)
    bqk_p = nc.declare_dram_parameter("bqk", [1, NFT * 128], F32R, isOutput=False)
    bv_p = nc.declare_dram_parameter("bv", [1, VW], F32R, isOutput=False)
    wp_p = nc.declare_dram_parameter("wp", [VW, E], F32R, isOutput=False)
    y_p = nc.declare_dram_parameter("y", [S, E], F32, isOutput=True)

    with tile.TileContext(nc) as tc:
        from contextlib import ExitStack

        with ExitStack() as outer:
            const = outer.enter_context(tc.tile_pool(name="const", bufs=1))
            stats = outer.enter_context(tc.tile_pool(name="stats", bufs=24))
            v_pool = outer.enter_context(tc.tile_pool(name="v_pool", bufs=1))
            oT_pool = outer.enter_context(tc.tile_pool(name="oT_pool", bufs=1))
            dram = outer.enter_context(tc.tile_pool(name="dram", bufs=1, space="DRAM"))

            ident = const.tile([128, 128], BF16)
            make_identity(nc, ident[:])
            ones_f32 = const.tile([1, 512], F32)
            nc.any.memset(ones_f32[:], 1.0)
            ones = const.tile([1, 512], F32R)
            nc.vector.tensor_copy(ones[:], ones_f32[:])
            bqk_sb = const.tile([1, NFT * 128], F32R)
            nc.sync.dma_start(bqk_sb[:], bqk_p[:])
            bv_sb = const.tile([1, VW], F32R)
            nc.sync.dma_start(bv_sb[:], bv_p[:])

            v_sb = [
                v_pool.tile([128, VW], BF16, name=f"vsb{st}", tag=f"vsb{st}")
                for st in range(NST)
            ]
            oT_sb = [
                oT_pool.tile([128, S], F32R, name=f"ot{h}", tag=f"ot{h}")
                for h in range(HLOC)
            ]
            qk_dram = dram.tile([NFT, 128, S], F32R, name="qkdram")

            # ---------------- Phase 1: QKV ----------------
            with ExitStack() as ph1:
                xt_pool = ph1.enter_context(tc.tile_pool(name="xt_pool", bufs=32))
                wv_pool = ph1.enter_context(tc.tile_pool(name="wv_pool", bufs=1))
                wqk_pool = ph1.enter_context(tc.tile_pool(name="wqk_pool", bufs=2))
                qkst_pool = ph1.enter_context(tc.tile_pool(name="qkst_pool", bufs=4))
                psv = ph1.enter_context(tc.tile_pool(name="psv", bufs=4, space="PSUM"))
                psq = ph1.enter_context(tc.tile_pool(name="psq", bufs=2, space="PSUM"))

                wv_sb = [
                    wv_pool.tile([128, VW], F32R, name=f"wvsb{c}", tag=f"wvsb{c}")
                    for c in range(NEC)
                ]
                for c in range(NEC):
                    nc.sync.dma_start(wv_sb[c][:], wv_p[c * 128:(c + 1) * 128, :])

                for sb in range(NSB):
                    xts = []
                    for c in range(NEC):
                        t = xt_pool.tile([128, 512], F32R, name="xt_t", tag="xt_t")
                        nc.sync.dma_start(
                            t[:], xt_p[c * 128:(c + 1) * 128, sb * 512:(sb + 1) * 512]
                        )
                        xts.append(t)

                    # v part: psum_v[st] = x_blk.T @ wv (+ bv)
                    ps_v = [
                        psv.tile([128, VW], F32, name="ps_v", tag="ps_v")
                        for _ in range(4)
                    ]
                    for c in range(NEC):
                        for st in range(4):
                            nc.tensor.matmul(
                                ps_v[st][:],
                                xts[c][:, st * 128:(st + 1) * 128],
                                wv_sb[c][:],
                                start=(c == 0),
                                stop=False,
                            )
                    for st in range(4):
                        nc.tensor.matmul(
                            ps_v[st][:],
                            ones[:1, :128],
                            bv_sb[:1, :],
                            start=False,
                            stop=True,
                        )
                        nc.any.tensor_copy(v_sb[sb * 4 + st][:], ps_v[st][:])

                    # qk part: psum_qk = wqk_tile.T @ x_blk (+ bqk)
                    for ft in range(NFT):
                        wqk_t = wqk_pool.tile([128, E], F32R, name="wqk_t", tag="wqk_t")
                        nc.sync.dma_start(wqk_t[:], wqk_p[ft])
                        ps_qk = psq.tile([128, 512], F32, name="ps_qk", tag="ps_qk")
                        for c in range(NEC):
                            nc.tensor.matmul(
                                ps_qk[:],
                                wqk_t[:, c * 128:(c + 1) * 128],
                                xts[c][:],
                                start=(c == 0),
                                stop=False,
                            )
                        nc.tensor.matmul(
                            ps_qk[:],
                            bqk_sb[:1, ft * 128:(ft + 1) * 128],
                            ones[:1, :512],
                            start=False,
                            stop=True,
                        )
                        qk_st = qkst_pool.tile([128, 512], F32R, name="qk_st", tag="qk_st")
                        nc.any.tensor_copy(qk_st[:], ps_qk[:])
                        nc.sync.dma_start(
                            qk_dram[ft, :, sb * 512:(sb + 1) * 512], qk_st[:]
                        )

            # ---------------- Phase 2: attention per head ----------------
            with ExitStack() as ph2:
                qk_pool = ph2.enter_context(tc.tile_pool(name="qk_pool", bufs=2))
                attn_pool = ph2.enter_context(tc.tile_pool(name="attn_pool", bufs=8))
                attnT_pool = ph2.enter_context(tc.tile_pool(name="attnT_pool", bufs=24))
                pss = ph2.enter_context(tc.tile_pool(name="pss", bufs=4, space="PSUM"))
                pst = ph2.enter_context(tc.tile_pool(name="pst", bufs=2, space="PSUM"))
                pso = ph2.enter_context(tc.tile_pool(name="pso", bufs=2, space="PSUM"))

                for h in range(HLOC):
                    qh = qk_pool.tile([128, S], F32R, name="qh", tag="qh")
                    nc.sync.dma_start(qh[:], qk_dram[2 * h])
                    kh = qk_pool.tile([128, S], F32R, name="kh", tag="kh")
                    nc.sync.dma_start(kh[:], qk_dram[2 * h + 1])

                    for qb in range(NQB):
                        attn_tiles = []
                        for qt in range(4):
                            qti = qb * 4 + qt
                            ps_s = [
                                pss.tile([128, 512], F32, name="ps_s", tag="ps_s")
                                for _ in range(NKB)
                            ]
                            for kb in range(NKB):
                                nc.tensor.matmul(
                                    ps_s[kb][:],
                                    qh[:, qti * 128:(qti + 1) * 128],
                                    kh[:, kb * 512:(kb + 1) * 512],
                                    start=True,
                                    stop=True,
                                )
                            mx = stats.tile([128, NKB], F32, name="mx", tag="mx")
                            for kb in range(NKB):
                                nc.vector.tensor_reduce(
                                    mx[:, kb:kb + 1], ps_s[kb][:], axis=AX.X, op=OP.max
                                )
                            negmax = stats.tile([128, 1], F32, name="negmax", tag="negmax")
                            nc.vector.tensor_reduce(
                                negmax[:], mx[:], axis=AX.X, op=OP.max, negate=True
                            )
                            attn_t = attn_pool.tile([128, S], BF16, name="attn_t", tag="attn_t")
                            sm = stats.tile([128, NKB], F32, name="sm", tag="sm")
                            for kb in range(NKB):
                                nc.scalar.activation(
                                    attn_t[:, kb * 512:(kb + 1) * 512],
                                    ps_s[kb][:],
                                    ACTF.Exp,
                                    bias=negmax[:],
                                    scale=1.0,
                                    accum_out=sm[:, kb:kb + 1],
                                )
                            sumx = stats.tile([128, 1], F32, name="sumx", tag="sumx")
                            nc.vector.tensor_reduce(sumx[:], sm[:], axis=AX.X, op=OP.add)
                            recip = stats.tile([128, 1], F32, name="recip", tag="recip")
                            nc.vector.reciprocal(recip[:], sumx[:])
                            nc.vector.tensor_scalar_mul(attn_t[:], attn_t[:], recip[:])
                            attn_tiles.append(attn_t)

                        attnT = []
                        for kt in range(NKT):
                            ps_t = pst.tile([128, 512], BF16, name="ps_t", tag="ps_t")
                            for qt in range(4):
                                nc.tensor.transpose(
                                    ps_t[:, qt * 128:(qt + 1) * 128],
                                    attn_tiles[qt][:, kt * 128:(kt + 1) * 128],
                                    ident[:],
                                )
                            at = attnT_pool.tile([128, 512], BF16, name="at", tag="at")
                            nc.any.tensor_copy(at[:], ps_t[:])
                            attnT.append(at)

                        ps_o = pso.tile([128, 512], F32, name="ps_o", tag="ps_o")
                        for kt in range(NKT):
                            nc.tensor.matmul(
                                ps_o[:],
                                v_sb[kt][:, h * 128:(h + 1) * 128],
                                attnT[kt][:],
                                start=(kt == 0),
                                stop=(kt == NKT - 1),
                            )
                        nc.any.tensor_copy(
                            oT_sb[h][:, qb * 512:(qb + 1) * 512], ps_o[:]
                        )

            # ---------------- Phase 3: proj (partial) ----------------
            with ExitStack() as ph3:
                wp_pool = ph3.enter_context(tc.tile_pool(name="wp_pool", bufs=8))
                y_pool = ph3.enter_context(tc.tile_pool(name="y_pool", bufs=4))
                psy = ph3.enter_context(tc.tile_pool(name="psy", bufs=4, space="PSUM"))

                for eb in range(NEB):
                    wp_ts = []
                    for r in range(HLOC):
                        w = wp_pool.tile([128, 512], F32R, name="wp_t", tag="wp_t")
                        nc.sync.dma_start(
                            w[:], wp_p[r * 128:(r + 1) * 128, eb * 512:(eb + 1) * 512]
                        )
                        wp_ts.append(w)
                    for qti in range(NQT):
                        ps_y = psy.tile([128, 512], F32, name="ps_y", tag="ps_y")
                        for h in range(HLOC):
                            nc.tensor.matmul(
                                ps_y[:],
                                oT_sb[h][:, qti * 128:(qti + 1) * 128],
                                wp_ts[h][:],
                                start=(h == 0),
                                stop=(h == HLOC - 1),
                            )
                        y_t = y_pool.tile([128, 512], F32, name="y_t", tag="y_t")
                        nc.any.tensor_copy(y_t[:], ps_y[:])
                        nc.sync.dma_start(
                            y_p[qti * 128:(qti + 1) * 128, eb * 512:(eb + 1) * 512],
                            y_t[:],
                        )

    _split_multi_waits(nc)
    return nc


# ---------------------------------------------------------------------------
# Host-side sharding / gather
# ---------------------------------------------------------------------------

def _prep_in_maps(query, Wqkv, bqkv, Wproj, S=S_FULL, HLOC=HLOC_FULL, n_cores=N_CORES):
    scale = np.float32(HD ** 0.5)
    groups = n_cores // B
    in_maps = []
    xt_cache = {}
    for c in range(n_cores):
        b, g = c // groups, c % groups
        heads = [g * HLOC + hh for hh in range(HLOC)]
        if b not in xt_cache:
            xt_cache[b] = np.ascontiguousarray(query[b][:S].T.astype(np.float32))
        NFT = 2 * HLOC
        wqk = np.empty((NFT, 128, E), dtype=np.float32)
        bqk = np.empty((NFT * 128,), dtype=np.float32)
        wv = np.empty((E, HLOC * 128), dtype=np.float32)
        bv = np.empty((HLOC * 128,), dtype=np.float32)
        wp = np.empty((HLOC * 128, E), dtype=np.float32)
        for hh, hd_ in enumerate(heads):
            base = hd_ * (3 * HD)
            wq = Wqkv[base:base + HD, :] * scale          # [128, E]
            wk = Wqkv[base + HD:base + 2 * HD, :]
            wvh = Wqkv[base + 2 * HD:base + 3 * HD, :]
            # [E,128] -> chunked [128, E] layout: arr[p, c*128+j] = W.T[c*128+p, j]
            wqk[2 * hh] = (
                wq.T.reshape(E // 128, 128, HD).transpose(1, 0, 2).reshape(128, E)
            )
            wqk[2 * hh + 1] = (
                wk.T.reshape(E // 128, 128, HD).transpose(1, 0, 2).reshape(128, E)
            )
            bqk[2 * hh * 128:(2 * hh + 1) * 128] = bqkv[base:base + HD] * scale
            bqk[(2 * hh + 1) * 128:(2 * hh + 2) * 128] = bqkv[base + HD:base + 2 * HD]
            wv[:, hh * 128:(hh + 1) * 128] = wvh.T
            bv[hh * 128:(hh + 1) * 128] = bqkv[base + 2 * HD:base + 3 * HD]
            wp[hh * 128:(hh + 1) * 128, :] = Wproj[:, hd_ * HD:(hd_ + 1) * HD].T
        in_maps.append(
            {
                "xt": xt_cache[b],
                "wqk": np.ascontiguousarray(wqk),
                "wv": np.ascontiguousarray(wv),
                "bqk": bqk.reshape(1, -1),
                "bv": bv.reshape(1, -1),
                "wp": np.ascontiguousarray(wp),
            }
        )
    return in_maps


_CACHE = {}


def _get_program(S=S_FULL, HLOC=HLOC_FULL):
    key = (S, HLOC)
    if key not in _CACHE:
        _CACHE[key] = build_program(S, HLOC)
    return _CACHE[key]


def run(query, Wqkv, bqkv, Wproj, bproj, trace=False, S=S_FULL, HLOC=HLOC_FULL,
        n_cores=N_CORES):
    from concourse.bass_utils import run_bass_kernel_spmd

    nc = _get_program(S, HLOC)
    in_maps = _prep_in_maps(query, Wqkv, bqkv, Wproj, S=S, HLOC=HLOC, n_cores=n_cores)
    res = run_bass_kernel_spmd(
        nc, in_maps, core_ids=list(range(n_cores)), trace=trace
    )
    groups = n_cores // B
    out = np.zeros((B, S, E), dtype=np.float32)
    for c in range(n_cores):
        out[c // groups] += res.results[c]["y"]
    out += bproj.astype(np.float32)
    return out, res


def kernel(**inputs):
    out, _ = run(
        np.asarray(inputs["query"], dtype=np.float32),
        np.asarray(inputs["Wqkv"], dtype=np.float32),
        np.asarray(inputs["bqkv"], dtype=np.float32),
        np.asarray(inputs["Wproj"], dtype=np.float32),
        np.asarray(inputs["bproj"], dtype=np.float32),
        trace=False,
    )
    return out
